# revision 1
# baseline (speedup 1.0000x reference)
"""Trainium2 Bass kernel for the Exprnn-style model (nn_Exprnn_2542620639651).

Pipeline: enc MLP (2x relu) -> orthogonal RNN with modrelu over T=512 ->
linear decoder.  Sharding: pure data parallel over batch (8 cores x 1024).

Instead of 512 serial matmul steps, the recurrence is solved by a
fixed-point linear-scan decomposition.  modrelu(z) = z + d(z) with
|d| <= |mb| = 0.01 always, so  h_t = sum_{k<=t} (u_k + d_k) R^{t-k}  is a
LINEAR scan over v = u + d plus a tiny correction stream d:

  scan 1:  h~_t = linear_scan(u)            (d = 0)
  extract: dd_t = -(modrelu(h~_t) - h~_t)   (parallel elementwise)
  scan 2:  out  = decode(linear_scan(u + d))

Each scan runs as 57 blocks of TB=9 timesteps (T padded 512->513).  Time
lives on SBUF partitions (10j+r for timestep-in-block j, hidden r), batch
(1024) on the free dim.  A block is ONE triangular block matmul with
constant weights  Win @ R^(j-k)  (+ a carry matmul  R^(j+1)  against the
previous block's last state, + a dd matmul in scan 2), all accumulated in
PSUM rows 0..89; rows 96..105 hold the carry (h at block end) produced by
extra lhsT columns, evicted with a partition-shifting copy to SBUF rows
0..9 for the next block's carry matmul.  The decoder (W3@W4) is folded
into scan 2's weights.  The only serial dependency left is the 57-step
carry chain per scan.

Validated end-to-end vs the fp32 reference at ~4e-3 max rel err with
realistic dtypes (bf16 x2/dd/A/B weights, f32r carry matmuls, fp32 PSUM).
"""

import os
import sys
from contextlib import ExitStack

for _p in ("/root/.axon_site/_ro/trn_rl_repo", "/opt/trn_rl_repo"):
    if os.path.isdir(_p) and _p not in sys.path:
        sys.path.append(_p)

import numpy as np
import ml_dtypes

import concourse.bass as bass
import concourse.tile as tile
from concourse import bacc, mybir
from concourse.bass_utils import run_bass_kernel_spmd

dt = mybir.dt
Alu = mybir.AluOpType
Act = mybir.ActivationFunctionType

# Problem shape (hardcoded per contract)
B, T, NI, H = 8192, 512, 2, 10
NCORES = 8
NB = B // NCORES          # 1024 batch per core = free dim
TB = 9                    # timesteps per scan block
NBLK = 57                 # blocks (57*9 = 513, time padded with zeros)
TPAD = TB * NBLK
KA = 10 * TB              # 90: x2/dd contraction partitions (outputs rows 0..89)
CO = 96                   # carry-row base in PSUM (32-aligned read); evicted to SBUF rows 0..9
M = CO + 10               # 106: psum rows = outputs(0:90) + pad + carry(96:106)
KX = NI * TB              # 12: encoder-input partitions
S = 2                     # column streams (matmul moving dim = NB/S = 512)
NS = NB // S
KBIG = float(2.0 ** 40)

_cache = {}


def _build_program():
    nc = bacc.Bacc("TRN2", target_bir_lowering=False, debug=False)
    f32, f32r, bf16 = dt.float32, dt.float32r, dt.bfloat16
    global bf16_

    bf16_ = bf16
    xin = nc.dram_tensor("xin", [NBLK, KA, NB], bf16_, kind="ExternalInput").ap()
    dlw2 = nc.dram_tensor("lw2", [KA, KA], bf16_, kind="ExternalInput").ap()
    da1 = nc.dram_tensor("a1", [KA, M], bf16, kind="ExternalInput").ap()
    da2 = nc.dram_tensor("a2", [KA, M], bf16, kind="ExternalInput").ap()
    db2w = nc.dram_tensor("b2w", [KA, M], bf16, kind="ExternalInput").ap()
    dc1 = nc.dram_tensor("c1w", [10, M], f32r, kind="ExternalInput").ap()
    dc2 = nc.dram_tensor("c2w", [10, M], f32r, kind="ExternalInput").ap()
    db2t = nc.dram_tensor("b2t", [KA, 1], f32, kind="ExternalInput").ap()
    dcmul = nc.dram_tensor("cmul", [KA, 1], f32, kind="ExternalInput").ap()
    dchi = nc.dram_tensor("chi", [KA, 1], f32, kind="ExternalInput").ap()
    dclo = nc.dram_tensor("clo", [KA, 1], f32, kind="ExternalInput").ap()
    yout = nc.dram_tensor("yout", [NBLK, KA, NB], f32, kind="ExternalOutput").ap()

    with tile.TileContext(nc) as tc, ExitStack() as ctx:
        wp = ctx.enter_context(tc.tile_pool(name="weights", bufs=1))
        xp = ctx.enter_context(tc.tile_pool(name="xin", bufs=3))
        x2p = ctx.enter_context(tc.tile_pool(name="x2", bufs=6))
        zp = ctx.enter_context(tc.tile_pool(name="zt", bufs=3))
        ep = ctx.enter_context(tc.tile_pool(name="et", bufs=3))
        ddp = ctx.enter_context(tc.tile_pool(name="dd", bufs=4))
        c1p = ctx.enter_context(tc.tile_pool(name="car1", bufs=2))
        c2p = ctx.enter_context(tc.tile_pool(name="car2", bufs=2))
        otp = ctx.enter_context(tc.tile_pool(name="ot", bufs=3))
        sps = ctx.enter_context(tc.tile_pool(name="scanps", bufs=4, space="PSUM"))

        def wtile(name, dram, shape, dtype, rows=None):
            t = wp.tile(shape, dtype, tag=name)
            nc.sync.dma_start(t[rows, :] if rows else t[:], dram[:])
            return t

        lw2 = wtile("lw2", dlw2, [KA, KA], bf16)
        a1 = wtile("a1", da1, [KA, M], bf16)
        a2 = wtile("a2", da2, [KA, M], bf16)
        b2w = wtile("b2w", db2w, [KA, M], bf16)
        c1w = wtile("c1w", dc1, [10, M], f32r)
        c2w = wtile("c2w", dc2, [10, M], f32r)
        b2t = wtile("b2t", db2t, [KA, 1], f32)
        cmul = wtile("cmul", dcmul, [KA, 1], f32)
        chi = wtile("chi", dchi, [KA, 1], f32)
        clo = wtile("clo", dclo, [KA, 1], f32)

        car1 = car2 = None
        NH = NB // 2
        for b in range(NBLK):
            # ---- encoder layer 2 (enc1 folded into host prep) ----
            xt = xp.tile([KA, NB], bf16)
            nc.sync.dma_start(xt[:], xin[b])
            x2t = x2p.tile([KA, NB], bf16)
            ps = sps.tile([M, NB], f32, tag="scan")
            nc.tensor.matmul(ps[:KA, :NH], lw2[:], xt[:, :NH], start=True, stop=True)
            nc.tensor.matmul(ps[:KA, NH:], lw2[:], xt[:, NH:], start=True, stop=True)
            nc.scalar.activation(x2t[:], ps[:KA, :], Act.Relu, bias=b2t[:])

            # ---- scan 1: h~ block + carry chain ----
            zt = zp.tile([KA, NB], bf16)
            ncar1 = c1p.tile([10, NB], f32r)
            ps = sps.tile([M, NB], f32, tag="scan")
            nc.tensor.matmul(ps[:, :NH], a1[:], x2t[:, :NH], start=True, stop=(b == 0))
            nc.tensor.matmul(ps[:, NH:], a1[:], x2t[:, NH:], start=True, stop=(b == 0))
            if b > 0:
                nc.tensor.matmul(ps[:, :NH], c1w[:], car1[:, :NH],
                                 start=False, stop=True, skip_group_check=True)
                nc.tensor.matmul(ps[:, NH:], c1w[:], car1[:, NH:],
                                 start=False, stop=True, skip_group_check=True)
            # z~ eviction (bf16) on ACT; carry eviction shifted to rows 0..9 on DVE
            nc.scalar.activation(zt[:], ps[:KA, :], Act.Copy)
            nc.vector.tensor_copy(ncar1[:, :NH], ps[CO:M, :NH])
            nc.scalar.activation(ncar1[:, NH:], ps[CO:M, NH:], Act.Copy)
            car1 = ncar1

            # ---- dd extraction on DVE (bf16 4x mode) ----
            # dd_neg = max(min(z*c, |mb|), -|mb|)   (c = 1 or -2^40 per row)
            et = ep.tile([KA, NB], bf16)
            ddt = ddp.tile([KA, NB], bf16)
            nc.vector.tensor_scalar(et[:], zt[:], cmul[:], chi[:],
                                    Alu.mult, Alu.min)
            nc.vector.tensor_scalar(ddt[:], et[:], clo[:], None, Alu.max)

            # ---- scan 2: decoded output + its own carry chain ----
            ot = otp.tile([KA, NB], f32)
            ncar2 = c2p.tile([10, NB], f32r)
            ps = sps.tile([M, NB], f32, tag="scan")
            nc.tensor.matmul(ps[:, :NH], a2[:], x2t[:, :NH], start=True, stop=False)
            nc.tensor.matmul(ps[:, NH:], a2[:], x2t[:, NH:], start=True, stop=False)
            nc.tensor.matmul(ps[:, :NH], b2w[:], ddt[:, :NH], start=False, stop=(b == 0))
            nc.tensor.matmul(ps[:, NH:], b2w[:], ddt[:, NH:], start=False, stop=(b == 0))
            if b > 0:
                nc.tensor.matmul(ps[:, :NH], c2w[:], car2[:, :NH],
                                 start=False, stop=True, skip_group_check=True)
                nc.tensor.matmul(ps[:, NH:], c2w[:], car2[:, NH:],
                                 start=False, stop=True, skip_group_check=True)
            # output eviction on ACT; carry eviction on DVE
            nc.scalar.activation(ot[:], ps[:KA, :], Act.Copy)
            nc.vector.tensor_copy(ncar2[:, :NH], ps[CO:M, :NH])
            nc.scalar.activation(ncar2[:, NH:], ps[CO:M, NH:], Act.Copy)
            car2 = ncar2
            nc.sync.dma_start(yout[b], ot[:])

    nc.compile()
    return nc


def _prep_inputs(inputs):
    X = np.ascontiguousarray(inputs["X"], dtype=np.float32)
    W1, b1v, W2, b2v = (np.asarray(inputs[k], np.float64) for k in ("W1", "b1", "W2", "b2"))
    Win, R, mbv = (np.asarray(inputs[k], np.float64) for k in ("Win", "R", "mb"))
    W3, b3v, W4, b4v = (np.asarray(inputs[k], np.float64) for k in ("W3", "b3", "W4", "b4"))
    Dm = W3 @ W4
    c4 = (b3v @ W4 + b4v).astype(np.float32)

    Rp = [np.eye(H)]
    for _ in range(TB + 1):
        Rp.append(Rp[-1] @ R)

    def blockdiag(Mx, reps):
        K, Ho = Mx.shape
        out = np.zeros((K * reps, Ho * reps), np.float32)
        for i in range(reps):
            out[i * K:(i + 1) * K, i * Ho:(i + 1) * Ho] = Mx
        return out

    def lhsA(dec):
        L = np.zeros((KA, M), np.float64)
        for k in range(TB):
            for j in range(k, TB):
                blk = Win @ Rp[j - k]
                L[10 * k:10 * k + 10, 10 * j:10 * j + 10] = blk @ Dm if dec else blk
            L[10 * k:10 * k + 10, CO:] = Win @ Rp[TB - 1 - k]
        return L

    def lhsB(dec):
        L = np.zeros((KA, M), np.float64)
        for k in range(TB):
            for j in range(k, TB):
                blk = Rp[j - k]
                L[10 * k:10 * k + 10, 10 * j:10 * j + 10] = -(blk @ Dm) if dec else -blk
            L[10 * k:10 * k + 10, CO:] = -Rp[TB - 1 - k]
        return L

    def lhsC(dec):
        L = np.zeros((10, M), np.float64)
        for j in range(TB):
            blk = Rp[j + 1]
            L[:, 10 * j:10 * j + 10] = blk @ Dm if dec else blk
        L[:, CO:] = Rp[TB]
        return L

    # host enc1 (1% of model FLOPs): x1 = relu(X@W1+b1), zero-padded T -> TPAD,
    # reshaped to [core, block, 10j+r, n], bf16
    x1 = np.maximum(X @ W1.astype(np.float32) + b1v.astype(np.float32), 0)
    Xc = x1.reshape(NCORES, NB, T, H)
    Xp = np.zeros((NCORES, NB, TPAD, H), np.float32)
    Xp[:, :, :T] = Xc
    Xin = np.ascontiguousarray(
        Xp.reshape(NCORES, NB, NBLK, TB, H).transpose(0, 2, 3, 4, 1)
        .reshape(NCORES, NBLK, KA, NB).astype(ml_dtypes.bfloat16)
    )

    mbt = np.tile(mbv, TB).astype(np.float32)
    shared = {
        "lw2": blockdiag(W2, TB).astype(ml_dtypes.bfloat16),
        "a1": lhsA(False).astype(ml_dtypes.bfloat16),
        "a2": lhsA(True).astype(ml_dtypes.bfloat16),
        "b2w": lhsB(True).astype(ml_dtypes.bfloat16),
        "c1w": lhsC(False).astype(np.float32),
        "c2w": lhsC(True).astype(np.float32),
        "b2t": np.ascontiguousarray(np.tile(b2v, TB).astype(np.float32).reshape(KA, 1)),
        "cmul": np.ascontiguousarray(np.where(mbt <= 0, 1.0, -KBIG).astype(np.float32).reshape(KA, 1)),
        "chi": np.ascontiguousarray(np.abs(mbt).reshape(KA, 1)),
        "clo": np.ascontiguousarray((-np.abs(mbt)).reshape(KA, 1)),
    }
    in_maps = [dict(shared, xin=Xin[c]) for c in range(NCORES)]
    return in_maps, c4


def _gather(results, c4):
    out = np.empty((B, T, H), np.float32)
    for c in range(NCORES):
        yo = results[c]["yout"]  # [NBLK, KA, NB]
        full = yo.reshape(NBLK, TB, H, NB).transpose(3, 0, 1, 2).reshape(NB, TPAD, H)
        out[c * NB:(c + 1) * NB] = full[:, :T]
    if np.any(c4):
        out += c4
    return out


def kernel(**inputs):
    if "nc" not in _cache:
        _cache["nc"] = _build_program()
    in_maps, c4 = _prep_inputs(inputs)
    res = run_bass_kernel_spmd(_cache["nc"], in_maps, core_ids=list(range(NCORES)))
    return _gather(res.results, c4)



# revision 4
# speedup vs baseline: 1.7656x; 1.7656x over previous
"""Trainium2 Bass kernel for the Exprnn-style model (nn_Exprnn_2542620639651).

Pipeline: enc MLP (2x relu, hosted in prep) -> orthogonal RNN with modrelu
over T=512 -> linear decoder.  Sharding: pure data parallel over batch
(8 cores x 1024).

The recurrence is solved by a fixed-point linear-scan decomposition.
modrelu(z) = z + d(z) with |d| <= |mb| <= 0.01, so the scan splits into a
linear scan of u (h~), a parallel extraction of the correction stream d
from h~, and a corrected+decoded linear scan of (u + d):

  scan 1:  ps1 = a1 @ x2 (+ carry)            h~ blocks, undecoded
  extract: t  = ps1 * (c/|mb|)  (ACT, bf16)   per-row scaled copy
           dd = clip(t, -1, 1)  (GPSIMD)      == -d/|mb| per row
  scan 2:  ps2 = a2 @ x2 + b2w @ dd (+ carry) decoded, corrected output

Time lives on SBUF partitions (10j+r for timestep-in-block j, hidden r),
batch (1024) on the free dim; T padded 512 -> 517 = 47 blocks x TB=11.
Block-local time mixing is a constant triangular matrix (Win R^(j-k), with
the decoder D=W3@W4 folded into scan 2).  The serial dependency is only the
47-step carry chain per scan:
 - scan 1's carry is the last timestep's rows of the scaled eviction t
   (read at partition base 96, weights undo the scale), so it costs no
   extra eviction.
 - scan 2's carry uses 10 extra undecoded lhs columns (110..119), evicted
   f32 from PSUM partitions 96..120 (32-aligned) to the same partitions of
   an SBUF tile; the carry matmul's lhs lives at partition base 96 with
   zero rows for the 14 junk partitions.

Per-block engine budget at full PE clock: PE 10 matmuls ~2.1us (bound),
ACT two [*,1024] evictions ~1.9us, DVE one f32 carry ~1.3us, GPSIMD clip,
DMA 1.3us.  Scan 1 runs 3 blocks ahead of scan 2 so every PE dependency is
satisfied ~a full block early and the tensor engine never stalls (keeps
the p-state ramp at max clock).
"""

import os
import sys
from contextlib import ExitStack

for _p in ("/root/.axon_site/_ro/trn_rl_repo", "/opt/trn_rl_repo"):
    if os.path.isdir(_p) and _p not in sys.path:
        sys.path.append(_p)

import numpy as np
import ml_dtypes

import concourse.bass as bass
import concourse.tile as tile
from concourse import bacc, mybir
from concourse.bass_utils import run_bass_kernel_spmd

dt = mybir.dt
Alu = mybir.AluOpType
Act = mybir.ActivationFunctionType

# Problem shape (hardcoded per contract)
B, T, NI, H = 8192, 512, 2, 10
NCORES = 8
NB = B // NCORES          # 1024 batch per core = free dim
TB = 11                   # timesteps per scan block
NBLK = 47                 # blocks (47*11 = 517, time padded with zeros)
TPAD = TB * NBLK
KA = H * TB               # 110: contraction partitions / h~ output rows
M2 = KA + H               # 120: scan2 psum rows = outputs + carry cols
CB = 64                   # aligned partition base for carry-read matmul operands
NS = NB // 2              # 512: matmul moving dim per stream

_cache = {}


def _build_program():
    nc = bacc.Bacc("TRN2", target_bir_lowering=False, debug=False)
    f32, f32r, bf16 = dt.float32, dt.float32r, dt.bfloat16

    xin = nc.dram_tensor("xin", [NBLK, KA, NB], bf16, kind="ExternalInput").ap()
    da1 = nc.dram_tensor("a1", [KA, KA], bf16, kind="ExternalInput").ap()
    da2 = nc.dram_tensor("a2", [KA, M2], bf16, kind="ExternalInput").ap()
    db2 = nc.dram_tensor("b2w", [KA, M2], bf16, kind="ExternalInput").ap()
    dc1 = nc.dram_tensor("c1w", [KA - CB, KA], bf16, kind="ExternalInput").ap()
    dc2 = nc.dram_tensor("c2w", [M2 - CB, M2], f32r, kind="ExternalInput").ap()
    dcs = nc.dram_tensor("cs", [KA, 1], f32, kind="ExternalInput").ap()
    yout = nc.dram_tensor("yout", [NBLK, KA, NB], bf16, kind="ExternalOutput").ap()

    with tile.TileContext(nc) as tc, ExitStack() as ctx:
        wp = ctx.enter_context(tc.tile_pool(name="weights", bufs=1))
        xp = ctx.enter_context(tc.tile_pool(name="xin", bufs=6))
        tp = ctx.enter_context(tc.tile_pool(name="tt", bufs=2))
        ddp = ctx.enter_context(tc.tile_pool(name="dd", bufs=4))
        c2p = ctx.enter_context(tc.tile_pool(name="car2", bufs=2))
        otp = ctx.enter_context(tc.tile_pool(name="ot", bufs=3))
        sp1 = ctx.enter_context(tc.tile_pool(name="ps1", bufs=2, space="PSUM"))
        sp2 = ctx.enter_context(tc.tile_pool(name="ps2", bufs=2, space="PSUM"))

        a1 = wp.tile([KA, KA], bf16, tag="a1")
        nc.sync.dma_start(a1[:], da1[:])
        a2 = wp.tile([KA, M2], bf16, tag="a2")
        nc.sync.dma_start(a2[:], da2[:])
        b2w = wp.tile([KA, M2], bf16, tag="b2w")
        nc.sync.dma_start(b2w[:], db2[:])
        c1w = wp.tile([KA, KA], bf16, tag="c1w")
        nc.sync.dma_start(c1w[CB:KA, :], dc1[:])
        c2w = wp.tile([M2, M2], f32r, tag="c2w")
        nc.sync.dma_start(c2w[CB:M2, :], dc2[:])
        cs = wp.tile([KA, 1], f32, tag="cs")
        nc.sync.dma_start(cs[:], dcs[:])

        x2t = [None] * NBLK
        tt = [None] * NBLK
        ddt = [None] * NBLK
        ps1t = [None] * NBLK
        ps2t = [None] * NBLK
        car2 = [None] * NBLK
        ott = [None] * NBLK

        x2t[0] = xp.tile([KA, NB], bf16, tag="x2", name="x2t")
        nc.sync.dma_start(x2t[0][:], xin[0])

        for i in range(-3, NBLK):
            # prefetch x2 for scan1 of block i+4
            if 0 <= i + 4 < NBLK:
                j = i + 4
                x2t[j] = xp.tile([KA, NB], bf16, tag="x2", name="x2t")
                nc.sync.dma_start(x2t[j][:], xin[j])

            # t / dd extraction for block i+2 (ps1 completed last iteration)
            if 0 <= i + 2 < NBLK:
                j = i + 2
                tt[j] = tp.tile([KA, NB], bf16, tag="tt", name="tt")
                nc.scalar.activation(tt[j][:], ps1t[j][:KA, :], Act.Copy,
                                     scale=cs[:])
                ddt[j] = ddp.tile([KA, NB], bf16, tag="dd", name="ddt")
                nc.gpsimd.tensor_scalar(ddt[j][:], tt[j][:], 1.0, -1.0,
                                        Alu.min, Alu.max)

            # scan 2 of block i: decoded output + carry columns
            if i >= 0:
                ps2t[i] = ps2 = sp2.tile([M2, NB], f32, tag="ps2", name="ps2")
                nc.tensor.matmul(ps2[:, :NS], a2[:], x2t[i][:, :NS],
                                 start=True, stop=False, skip_group_check=True)
                nc.tensor.matmul(ps2[:, NS:], a2[:], x2t[i][:, NS:],
                                 start=True, stop=False, skip_group_check=True)
                nc.tensor.matmul(ps2[:, :NS], b2w[:], ddt[i][:, :NS],
                                 start=False, stop=(i == 0), skip_group_check=True)
                nc.tensor.matmul(ps2[:, NS:], b2w[:], ddt[i][:, NS:],
                                 start=False, stop=(i == 0), skip_group_check=True)
                if i > 0:
                    nc.tensor.matmul(ps2[:, :NS], c2w[CB:M2, :],
                                     car2[i - 1][CB:M2, :NS],
                                     start=False, stop=True, skip_group_check=True)
                    nc.tensor.matmul(ps2[:, NS:], c2w[CB:M2, :],
                                     car2[i - 1][CB:M2, NS:],
                                     start=False, stop=True, skip_group_check=True)

            # scan 1 of block i+3 (3 blocks ahead)
            if 0 <= i + 3 < NBLK:
                j = i + 3
                ps1t[j] = ps1 = sp1.tile([KA, NB], f32, tag="ps1", name="ps1")
                nc.tensor.matmul(ps1[:, :NS], a1[:], x2t[j][:, :NS],
                                 start=True, stop=(j == 0), skip_group_check=True)
                nc.tensor.matmul(ps1[:, NS:], a1[:], x2t[j][:, NS:],
                                 start=True, stop=(j == 0), skip_group_check=True)
                if j > 0:
                    nc.tensor.matmul(ps1[:, :NS], c1w[CB:KA, :],
                                     tt[j - 1][CB:KA, :NS],
                                     start=False, stop=True, skip_group_check=True)
                    nc.tensor.matmul(ps1[:, NS:], c1w[CB:KA, :],
                                     tt[j - 1][CB:KA, NS:],
                                     start=False, stop=True, skip_group_check=True)

            # evictions + output store for block i
            if i >= 0:
                ott[i] = ot = otp.tile([KA, NB], bf16, tag="ot", name="ot")
                nc.scalar.activation(ot[:], ps2t[i][:KA, :], Act.Copy)
                if i < NBLK - 1:
                    car2[i] = c2 = c2p.tile([M2, NB], f32r, tag="car2", name="car2")
                    nc.vector.tensor_copy(c2[CB:M2, :], ps2t[i][CB:M2, :])
                nc.sync.dma_start(yout[i], ot[:])

    nc.compile()
    return nc


def _prep_inputs(inputs):
    X = np.ascontiguousarray(inputs["X"], dtype=np.float32)
    W1, b1v, W2, b2v = (np.asarray(inputs[k], np.float64) for k in ("W1", "b1", "W2", "b2"))
    Win, R, mbv = (np.asarray(inputs[k], np.float64) for k in ("Win", "R", "mb"))
    W3, b3v, W4, b4v = (np.asarray(inputs[k], np.float64) for k in ("W3", "b3", "W4", "b4"))
    D = W3 @ W4
    c4 = (b3v @ W4 + b4v).astype(np.float32)

    Rp = [np.eye(H)]
    for _ in range(TB + 1):
        Rp.append(Rp[-1] @ R)

    cvec = np.where(mbv <= 0, 1.0, -(2.0 ** 20))
    mba = np.abs(mbv)

    def tri(f, cols):
        L = np.zeros((KA, cols), np.float64)
        for k in range(TB):
            for j in range(k, TB):
                L[10 * k:10 * k + 10, 10 * j:10 * j + 10] = f(k, j)
        return L

    a1 = tri(lambda k, j: Win @ Rp[j - k], KA)
    a2 = tri(lambda k, j: Win @ Rp[j - k] @ D, M2)
    b2w = tri(lambda k, j: -np.diag(mba) @ Rp[j - k] @ D, M2)
    for k in range(TB):
        a2[10 * k:10 * k + 10, KA:] = Win @ Rp[TB - 1 - k]
        b2w[10 * k:10 * k + 10, KA:] = -np.diag(mba) @ Rp[TB - 1 - k]

    # scan1 carry weights: rhs is t[64:110]; rows 64..99 are junk (earlier
    # timesteps) killed by zero weights, rows 100..109 carry
    # h~_end[r] * cvec[r]/|mb_r| which the weights undo.
    c1w = np.zeros((KA - CB, KA), np.float64)
    inv = mba / cvec
    for j in range(TB):
        c1w[KA - H - CB:, 10 * j:10 * j + 10] = np.diag(inv) @ Rp[j + 1]
    # scan2 carry weights: rhs is ps2[64:120]; rows 64..109 junk (decoded
    # outputs), rows 110..119 = undecoded h_end carry columns.
    c2w = np.zeros((M2 - CB, M2), np.float64)
    for j in range(TB):
        c2w[KA - CB:, 10 * j:10 * j + 10] = Rp[j + 1] @ D
    c2w[KA - CB:, KA:] = Rp[TB]

    cs = np.tile(cvec / mba, TB).astype(np.float32).reshape(KA, 1)

    # host encoder MLP (tiny 2->10->10), zero-padded T -> TPAD, reshaped to
    # [core, block, 10j+r, n], bf16
    x1 = np.maximum(X @ W1.astype(np.float32) + b1v.astype(np.float32), 0)
    x2 = np.maximum(x1 @ W2.astype(np.float32) + b2v.astype(np.float32), 0)
    Xc = x2.reshape(NCORES, NB, T, H)
    Xp = np.zeros((NCORES, NB, TPAD, H), np.float32)
    Xp[:, :, :T] = Xc
    Xin = np.ascontiguousarray(
        Xp.reshape(NCORES, NB, NBLK, TB * H).transpose(0, 2, 3, 1)
        .astype(ml_dtypes.bfloat16)
    )

    shared = {
        "a1": a1.astype(ml_dtypes.bfloat16),
        "a2": a2.astype(ml_dtypes.bfloat16),
        "b2w": b2w.astype(ml_dtypes.bfloat16),
        "c1w": c1w.astype(ml_dtypes.bfloat16),
        "c2w": c2w.astype(np.float32),
        "cs": np.ascontiguousarray(cs),
    }
    in_maps = [dict(shared, xin=Xin[c]) for c in range(NCORES)]
    return in_maps, c4


def _gather(results, c4):
    out = np.empty((B, T, H), np.float32)
    for c in range(NCORES):
        yo = results[c]["yout"]  # [NBLK, KA, NB] bf16
        full = (yo.astype(np.float32)
                .reshape(NBLK * TB, H, NB).transpose(2, 0, 1))
        out[c * NB:(c + 1) * NB] = full[:, :T]
    if np.any(c4):
        out += c4
    return out


def kernel(**inputs):
    if "nc" not in _cache:
        _cache["nc"] = _build_program()
    in_maps, c4 = _prep_inputs(inputs)
    res = run_bass_kernel_spmd(_cache["nc"], in_maps, core_ids=list(range(NCORES)))
    return _gather(res.results, c4)
